# revision 20
# baseline (speedup 1.0000x reference)
"""Distributed Trainium2 Bass kernel for causal GQA attention block.

Problem (hardcoded): x [4, 2048, 1024] f32; wq [1024, 1024]; wk/wv [1024, 256];
wo [1024, 1024]. 16 q-heads, 4 kv-heads, head_dim 64, rms-norm on q/k (no
weight), rope (base 10000), q gain 1.5, causal SDPA, out-proj.

Sharding over 8 cores: core i -> batch b = i//2, head-half p = i%2
(q-heads 8p..8p+7, kv-heads 2p, 2p+1 -- KV groups intact). Each core computes
its 8 heads' attention output O^T (feature-major), pairs AllGather O^T per
(q-tile, head-pair) slice, and each core computes a disjoint 512-column slice
of the out-projection.

On-chip layouts are feature-major ("transposed"): X^T, Q^T, K^T so the PE
contracts over partitions; V is token-major with a ones column appended so the
PV matmul also produces softmax row-sums (normalization happens on O^T).
Host pre-arranges x and weights into the SBUF layouts so all input DMAs are
contiguous copies.
"""
import sys

sys.path.insert(0, "/opt/trn_rl_repo")

import numpy as np
import ml_dtypes

import concourse.bacc as bacc
import concourse.mybir as mybir
import concourse.tile as tile
from concourse.bass_utils import run_bass_kernel_spmd

F32 = mybir.dt.float32
BF16 = mybir.dt.bfloat16
AF = mybir.ActivationFunctionType

N = 2048          # tokens
C = 1024          # model dim
DQ = 512          # local q out-features (8 heads x 64)
DKV = 128         # local kv out-features (2 kv heads x 64)
D = 64            # head dim
NCC = C // 128    # 8 contraction chunks
NQT = 4           # q tiles of 512
NTC = N // 128    # 16 token chunks
QK_GAIN = 1.5
ROPE_BASE = 10000.0
EXP_SCALE = QK_GAIN / np.sqrt(D).item()  # folded gain * 1/sqrt(D) = 0.1875
EPS = float(np.finfo(np.float32).eps)

# stream_shuffle mask: 32 groups of 4 partitions; rope rotation swaps
# 32-partition halves within each 64-partition head block.
ROT_MASK = list(range(8, 16)) + list(range(0, 8)) + list(range(24, 32)) + list(range(16, 24))


def _host_tables():
    inv_freq = (1.0 / (ROPE_BASE ** (np.arange(0, D, 2, dtype=np.float64) / D)))  # [32]
    t = np.arange(N, dtype=np.float64)
    ang = np.outer(inv_freq, t)  # [32, N]
    cos32 = np.cos(ang)
    sin32 = np.sin(ang)
    cosT = np.tile(cos32, (4, 1)).astype(np.float32)  # [128, N]
    sinTs = np.concatenate([-sin32, sin32, -sin32, sin32], axis=0).astype(np.float32)
    q = np.arange(128)
    trimask = (q[None, :] >= q[:, None]).astype(np.float32)  # keep q >= k
    ones33 = np.zeros((128, 33), np.float32)
    ones33[0:64, 0] = 1.0
    ones33[64:128, 32] = 1.0
    ident = np.eye(128, dtype=np.float32)
    exp33 = np.zeros((33, 64), np.float32)
    exp33[0, :] = 1.0
    exp33[32, :] = 1.0
    bf = ml_dtypes.bfloat16
    return {
        "cosT": cosT.astype(bf),
        "sinTs": sinTs.astype(bf),
        "trimask": trimask.astype(bf),
        "ones33": ones33.astype(bf),
        "ident": ident.astype(bf),
        "exp33": exp33.astype(bf),
    }


def build():
    nc = bacc.Bacc(None, target_bir_lowering=False, num_devices=8)

    # host pre-arranged: x^T token-tiled [qt, 128, cc, 512]; weights [128, cc, outfeat]
    x_ext = nc.declare_dram_parameter("x", [NQT, 128, NCC, 512], BF16, isOutput=False)
    wq_ext = nc.declare_dram_parameter("wq", [128, NCC, DQ], BF16, isOutput=False)
    wk_ext = nc.declare_dram_parameter("wk", [128, NCC, DKV], BF16, isOutput=False)
    wv_ext = nc.declare_dram_parameter("wv", [128, NCC, DKV], BF16, isOutput=False)
    wo_ext = nc.declare_dram_parameter("wo", [128, NCC, DQ], BF16, isOutput=False)
    out_ext = nc.declare_dram_parameter("out", [N, DQ], F32, isOutput=True)

    tabs = _host_tables()
    cosT_d = nc.inline_tensor(tabs["cosT"], name="cosT_d")
    sinTs_d = nc.inline_tensor(tabs["sinTs"], name="sinTs_d")
    trimask_d = nc.inline_tensor(tabs["trimask"], name="trimask_d")
    ones33_d = nc.inline_tensor(tabs["ones33"], name="ones33_d")
    ident_d = nc.inline_tensor(tabs["ident"], name="ident_d")
    exp33_d = nc.inline_tensor(tabs["exp33"], name="exp33_d")

    with tile.TileContext(nc) as tc:
        with (
            tc.tile_pool(name="dram", bufs=1, space="DRAM") as dram,
            tc.tile_pool(name="persist", bufs=1) as ps,
            tc.tile_pool(name="work", bufs=3) as wk,
        ):
            # ---- persistent SBUF tensors ----
            xT = ps.tile([128, NCC, N], BF16, name="xT")          # X^T chunks
            wq_sb = ps.tile([128, NCC, DQ], BF16, name="wq_sb")
            wk_sb = ps.tile([128, NCC, DKV], BF16, name="wk_sb")
            wv_sb = ps.tile([128, NCC, DKV], BF16, name="wv_sb")
            wo_sb = ps.tile([128, NCC, DQ], BF16, name="wo_sb")
            cosT = ps.tile([128, N], BF16, name="cosT")
            sinTs = ps.tile([128, N], BF16, name="sinTs")
            trimask = ps.tile([128, 128], BF16, name="trimask")
            ones33 = ps.tile([128, 33], BF16, name="ones33")
            ident = ps.tile([128, 128], BF16, name="ident")
            exp33 = ps.tile([33, 64], BF16, name="exp33")
            eps_sb = ps.tile([128, 1], F32, name="eps_sb")
            kTdA = ps.tile([128, N], BF16, name="kTdA")           # kv head A dup'd
            kTdB = ps.tile([128, N], BF16, name="kTdB")
            v_sb = ps.tile([128, NTC, 130], BF16, name="v_sb")    # [V_A|1|V_B|1]
            oT = ps.tile([128, 4, N], BF16, name="oT")            # own O^T (normed)

            # ---- phase A: stage inputs; x token-tiled so projections can
            # start after the first 1MB tile lands ----
            for qt in range(NQT):
                eng = nc.sync if qt % 2 == 0 else nc.scalar
                eng.dma_start(
                    out=xT[:, :, qt * 512:(qt + 1) * 512], in_=x_ext[qt],
                )
            nc.gpsimd.dma_start(out=wq_sb[:], in_=wq_ext[:])
            nc.gpsimd.dma_start(out=wk_sb[:], in_=wk_ext[:])
            nc.gpsimd.dma_start(out=ones33[:], in_=ones33_d[:])
            nc.gpsimd.dma_start(out=cosT[:], in_=cosT_d[:])
            nc.gpsimd.dma_start(out=sinTs[:], in_=sinTs_d[:])
            nc.gpsimd.dma_start(out=exp33[:], in_=exp33_d[:])
            nc.gpsimd.dma_start(out=wv_sb[:], in_=wv_ext[:])
            nc.gpsimd.dma_start(out=ident[:], in_=ident_d[:])
            nc.gpsimd.dma_start(out=trimask[:], in_=trimask_d[:])
            nc.gpsimd.dma_start(out=wo_sb[:], in_=wo_ext[:])
            nc.gpsimd.memset(eps_sb[:], EPS)
            nc.gpsimd.memset(v_sb[:, :, 64:65], 1.0)
            nc.gpsimd.memset(v_sb[:, :, 129:130], 1.0)

            # ---- unified psum/work pools (tags shared across phases for overlap) ----
            with (
                tc.tile_pool(name="u_psum", bufs=1, space="PSUM") as up,
                tc.tile_pool(name="u_sbuf", bufs=3) as bs,
            ):
                qT_raw = bs.tile([128, 4, N], BF16, name="qT_raw", bufs=1)
                kT_raw = bs.tile([128, N], BF16, name="kT_raw", bufs=1)

                def proj(w_ap, dst):
                    # dst [128, N]; w_ap [128, cc, 128] weight slice
                    for qp in range(NQT // 2):
                        pp = up.tile([128, 2, 512], F32, tag=("mm" if qp % 2 == 0 else "o"), bufs=2)
                        for h in range(2):
                            qt = 2 * qp + h
                            for cc in range(NCC):
                                nc.tensor.matmul(
                                    pp[:, h, :], w_ap[:, cc, :], xT[:, cc, qt * 512:(qt + 1) * 512],
                                    start=(cc == 0), stop=(cc == NCC - 1),
                                )
                        nc.vector.tensor_copy(
                            dst[:, qp * 1024:(qp + 1) * 1024],
                            pp.rearrange("p a b -> p (a b)"),
                        )

                def rope_sq(src):
                    sq = bs.tile([128, N], BF16, tag="sq", bufs=2)
                    nc.vector.tensor_mul(sq[:], src, src)
                    return sq

                def rope_ln(sq):
                    # rms denominators: rr2 rows 0/32 hold per-token 1/rms for A/B heads
                    lnv = bs.tile([33, N], F32, tag="lnv", bufs=1)
                    for qt in range(NQT):
                        msp = up.tile([33, 512], F32, tag="mm", bufs=2)
                        nc.tensor.matmul(
                            msp[:], ones33[:],
                            sq[:, qt * 512:(qt + 1) * 512], start=True, stop=True,
                        )
                        nc.scalar.activation(
                            lnv[:, qt * 512:(qt + 1) * 512], msp[:],
                            AF.Ln, bias=eps_sb[0:33, :], scale=1.0 / D,
                        )
                    rr2 = bs.tile([33, N], BF16, tag="rr2", bufs=2)
                    nc.scalar.activation(rr2[:], lnv[:], AF.Exp, scale=-0.5)
                    return rr2

                def rope_apply(src, rr2, dsts):
                    rot = bs.tile([128, N], BF16, tag="rot", bufs=2)
                    nc.vector.tensor_copy(rot[0:32, :], src[32:64, :])
                    nc.vector.tensor_copy(rot[32:64, :], src[0:32, :])
                    nc.vector.tensor_copy(rot[64:96, :], src[96:128, :])
                    nc.vector.tensor_copy(rot[96:128, :], src[64:96, :])
                    t1 = bs.tile([128, N], BF16, tag="t1", bufs=2)
                    nc.vector.tensor_mul(t1[:], src, cosT[:])
                    nc.vector.tensor_mul(rot[:], rot[:], sinTs[:])
                    nc.vector.tensor_add(t1[:], t1[:], rot[:])
                    # apply r (per head, per token) via expander broadcast
                    for qt in range(NQT):
                        rbp = up.tile([128, 512], F32, tag="mm", bufs=2)
                        nc.tensor.matmul(
                            rbp[0:64, :], exp33[0:1, :], rr2[0:1, qt * 512:(qt + 1) * 512],
                            start=True, stop=True,
                        )
                        nc.tensor.matmul(
                            rbp[64:128, :], exp33[32:33, :], rr2[32:33, qt * 512:(qt + 1) * 512],
                            start=True, stop=True,
                        )
                        for dst in dsts:
                            nc.vector.tensor_mul(
                                dst[:, qt * 512:(qt + 1) * 512],
                                t1[:, qt * 512:(qt + 1) * 512], rbp[:],
                            )

                vT = bs.tile([128, N], BF16, tag="og", bufs=2)

                def emit_vtrans(group):
                    # transpose V token-chunks 4*group .. 4*group+3 into v_sb
                    for tcix in range(4 * group, 4 * group + 4):
                        pv = up.tile([128, 128], BF16, tag=("mm" if tcix % 2 == 0 else "o"), bufs=2)
                        nc.tensor.transpose(pv[:], vT[:, tcix * 128:(tcix + 1) * 128], ident[:])
                        nc.vector.tensor_copy(v_sb[:, tcix, 0:64], pv[:, 0:64])
                        nc.vector.tensor_copy(v_sb[:, tcix, 65:129], pv[:, 64:128])

                # ---- phase D: attention (j outer) + per-(j,m) AllGather + out-proj ----
                qTf = qT_raw
                cc_ins = [[dram.tile([128, 512], BF16, name=f"cc_in{j}_{m}") for m in range(4)] for j in range(NQT)]
                cc_outs = [[dram.tile([2, 128, 512], BF16, name=f"cc_out{j}_{m}") for m in range(4)] for j in range(NQT)]
                og_tiles = {}

                def emit_outproj(jo, tts):
                    og = og_tiles[jo]
                    for tt in tts:
                        tcix = jo * 4 + tt
                        po = up.tile([128, 512], F32, tag="o", bufs=2, name=f"po{jo}_{tt}")
                        for rc in range(8):
                            nc.tensor.matmul(
                                po[:], og[:, rc, tt * 128:(tt + 1) * 128],
                                wo_sb[:, rc, :], start=(rc == 0), stop=(rc == 7),
                            )
                        ev = bs.tile([128, 512], F32, tag="ev", bufs=2, name=f"ev{jo}_{tt}")
                        nc.vector.tensor_copy(ev[:], po[:])
                        nc.sync.dma_start(
                            out=out_ext[tcix * 128:(tcix + 1) * 128, :], in_=ev[:]
                        )

                def emit_attention(j, m, defer_pv=False):
                    kT = kTdA if m < 2 else kTdB
                    vslot = 0 if m < 2 else 65
                    oab = up.tile([65, 2, 512], F32, tag="o", bufs=2)
                    nkc = 4 * (j + 1)

                    def emit_scores(kc):
                        i = kc - 4 * j
                        off = max(0, 128 * i)
                        w = 512 - off
                        q0 = 512 * j + off
                        sAB = up.tile([128, 2, 512], F32, tag="mm", bufs=2, name=f"sAB{kc}")
                        nc.tensor.matmul(
                            sAB[:, 0, 0:w], kT[0:64, kc * 128:(kc + 1) * 128],
                            qTf[0:64, m, q0:q0 + w], start=True, stop=True,
                            tile_position=(0, 0),
                        )
                        nc.tensor.matmul(
                            sAB[:, 1, 0:w], kT[64:128, kc * 128:(kc + 1) * 128],
                            qTf[64:128, m, q0:q0 + w], start=True, stop=True,
                            tile_position=(64, 0),
                        )
                        pAB = bs.tile([128, 2, 512], BF16, tag="pAB", bufs=5, name=f"pAB{kc}")
                        nc.scalar.activation(
                            pAB[:, :, 0:w], sAB[:, :, 0:w], AF.Exp, scale=EXP_SCALE
                        )
                        if i >= 0:
                            nc.vector.tensor_mul(
                                pAB[:, :, 0:128], pAB[:, :, 0:128],
                                trimask.rearrange("p (a b) -> p a b", a=1).broadcast_to([128, 2, 128]),
                            )
                        return pAB

                    def emit_pv(kc, pAB):
                        i = kc - 4 * j
                        off = max(0, 128 * i)
                        w = 512 - off
                        nc.tensor.matmul(
                            oab[:, 0, off:512], v_sb[:, kc, vslot:vslot + 65],
                            pAB[:, 0, 0:w], start=(kc == 0), stop=(kc == nkc - 1),
                            skip_group_check=True,
                        )
                        nc.tensor.matmul(
                            oab[:, 1, off:512], v_sb[:, kc, vslot:vslot + 65],
                            pAB[:, 1, 0:w], start=(kc == 0), stop=(kc == nkc - 1),
                            skip_group_check=True,
                        )

                    if defer_pv:
                        # emit all scores+exp now; PV + normalization resume
                        # via the returned closure (lets V transposes slot in
                        # between on the PE stream)
                        pabs = [emit_scores(kc) for kc in range(nkc)]

                        def finish():
                            for kc in range(nkc):
                                emit_pv(kc, pabs[kc])
                            _norm_and_gather(j, m, oab)
                        return finish
                    staged = []
                    for kc in range(nkc):
                        staged.append((kc, emit_scores(kc)))
                        if len(staged) == 2:
                            for kcx, px in staged:
                                emit_pv(kcx, px)
                            staged = []
                    for kcx, px in staged:
                        emit_pv(kcx, px)
                    _norm_and_gather(j, m, oab)

                def _norm_and_gather(j, m, oab):
                    # normalize from PSUM directly: r = 1/rowsum, broadcast,
                    # multiply into oT (psum-direct, no staging copies)
                    ssum = bs.tile([1, 1024], F32, tag="ssum", bufs=2)
                    nc.vector.tensor_copy(ssum[:], oab[64:65, :, :])
                    rrf = bs.tile([1, 1024], F32, tag="rrf", bufs=2)
                    nc.vector.reciprocal_approx_fast(rrf[:], ssum[:])
                    rrb = bs.tile([1, 1024], BF16, tag="rrb", bufs=2)
                    nc.vector.tensor_copy(rrb[:], rrf[:])
                    rbs = bs.tile([64, 1024], BF16, tag="rbs", bufs=2)
                    nc.gpsimd.partition_broadcast(rbs[:], rrb[:], channels=64)
                    nc.vector.tensor_mul(
                        oT[0:64, m, 512 * j:512 * (j + 1)], oab[0:64, 0, :], rbs[:, 0:512]
                    )
                    nc.vector.tensor_mul(
                        oT[64:128, m, 512 * j:512 * (j + 1)], oab[0:64, 1, :], rbs[:, 512:1024]
                    )
                    # per-(j,m) AllGather of this O^T slice
                    nc.sync.dma_start(
                        out=cc_ins[j][m][:], in_=oT[:, m, 512 * j:512 * (j + 1)]
                    )
                    nc.gpsimd.collective_compute(
                        "AllGather",
                        mybir.AluOpType.bypass,
                        replica_groups=[[0, 1], [2, 3], [4, 5], [6, 7]],
                        ins=[cc_ins[j][m].opt()],
                        outs=[cc_outs[j][m].opt()],
                    )
                    if m == 0:
                        og_tiles[j] = bs.tile([128, 8, 512], BF16, tag="og", bufs=2, name=f"og{j}")
                    og = og_tiles[j]
                    for r in range(2):
                        nc.sync.dma_start(
                            out=og[:, r * 4 + m, :], in_=cc_outs[j][m][r],
                        )

                # Head: Q-proj m0 + K-proj first so scalar (rms ln, then
                # attention exp) is fed as early as possible. Attention (0,0)
                # scores+exp run before V-proj (its PV resumes after the
                # first 4 V transposes).
                proj(wq_sb[:, :, 0:128], qT_raw[:, 0, :])
                proj(wk_sb, kT_raw[:])
                sq_q0 = rope_sq(qT_raw[:, 0, :])
                rr2_q0 = rope_ln(sq_q0)
                sq_k = rope_sq(kT_raw[:])
                rr2_k = rope_ln(sq_k)
                rope_apply(qT_raw[:, 0, :], rr2_q0, [qT_raw[:, 0, :]])
                rope_apply(kT_raw[:], rr2_k, [kTdA[:]])
                nc.vector.tensor_copy(kTdB[0:64, :], kTdA[64:128, :])
                nc.vector.tensor_copy(kTdB[64:128, :], kTdA[64:128, :])
                nc.vector.tensor_copy(kTdA[64:128, :], kTdA[0:64, :])
                att00_finish = emit_attention(0, 0, defer_pv=True)
                proj(wv_sb, vT[:])
                emit_vtrans(0)
                att00_finish()
                # j=0 attention interleaved with remaining Q proj + rope; V
                # transposes slot between rope_ln and rope_apply to cover the
                # rms scalar chain latency on the PE stream.
                for m in range(1, 4):
                    proj(wq_sb[:, :, m * 128:(m + 1) * 128], qT_raw[:, m, :])
                    sq = rope_sq(qT_raw[:, m, :])
                    rr2 = rope_ln(sq)
                    emit_vtrans(m)
                    rope_apply(qT_raw[:, m, :], rr2, [qT_raw[:, m, :]])
                    emit_attention(0, m)
                for j in range(1, NQT):
                    for m in range(4):
                        emit_attention(j, m)
                        emit_outproj(j - 1, [m])
                emit_outproj(NQT - 1, range(4))

    nc.finalize()
    return nc


_NC_CACHE = None


def _get_nc():
    global _NC_CACHE
    if _NC_CACHE is None:
        _NC_CACHE = build()
    return _NC_CACHE


def _make_in_maps(inputs):
    x = np.asarray(inputs["x"], dtype=np.float32)
    wq = np.asarray(inputs["wq"], dtype=np.float32)
    wk = np.asarray(inputs["wk"], dtype=np.float32)
    wv = np.asarray(inputs["wv"], dtype=np.float32)
    wo = np.asarray(inputs["wo"], dtype=np.float32)
    bf = ml_dtypes.bfloat16

    def warr(w):  # [1024, F] -> [128, 8, F]
        return np.ascontiguousarray(w.reshape(NCC, 128, -1).transpose(1, 0, 2).astype(bf))

    in_maps = []
    for i in range(8):
        b, p = i // 2, i % 2
        xb = x[b]  # [N, C]
        xT = xb.T.reshape(NCC, 128, N).transpose(1, 0, 2)  # [128, cc, n]
        x4 = np.ascontiguousarray(
            xT.reshape(128, NCC, NQT, 512).transpose(2, 0, 1, 3).astype(bf)
        )  # [qt, 128, cc, 512]
        in_maps.append({
            "x": x4,
            "wq": warr(wq[:, p * DQ:(p + 1) * DQ]),
            "wk": warr(wk[:, p * DKV:(p + 1) * DKV]),
            "wv": warr(wv[:, p * DKV:(p + 1) * DKV]),
            "wo": warr(wo[:, p * DQ:(p + 1) * DQ]),
        })
    return in_maps


def kernel(x, wq, wk, wv, wo):
    x = np.asarray(x, dtype=np.float32)
    B = x.shape[0]
    nc = _get_nc()
    in_maps = _make_in_maps({"x": x, "wq": wq, "wk": wk, "wv": wv, "wo": wo})
    res = run_bass_kernel_spmd(nc, in_maps, core_ids=list(range(8)))
    out = np.empty((B, N, C), dtype=np.float32)
    for b in range(B):
        out[b, :, 0:DQ] = res.results[2 * b]["out"]
        out[b, :, DQ:C] = res.results[2 * b + 1]["out"]
    return out


if __name__ == "__main__":
    rng = np.random.default_rng(0)
    ins = {
        "x": rng.standard_normal((4, N, C), dtype=np.float32),
        "wq": (rng.standard_normal((C, C), dtype=np.float32) * 0.02),
        "wk": (rng.standard_normal((C, 256), dtype=np.float32) * 0.02),
        "wv": (rng.standard_normal((C, 256), dtype=np.float32) * 0.02),
        "wo": (rng.standard_normal((C, C), dtype=np.float32) * 0.02),
    }
    y = kernel(**ins)
    print("out", y.shape, y.dtype, np.abs(y).mean())


# revision 25
# speedup vs baseline: 1.0918x; 1.0918x over previous
"""Distributed Trainium2 Bass kernel for causal GQA attention block.

Problem (hardcoded): x [4, 2048, 1024] f32; wq [1024, 1024]; wk/wv [1024, 256];
wo [1024, 1024]. 16 q-heads, 4 kv-heads, head_dim 64, rms-norm on q/k (no
weight), rope (base 10000), q gain 1.5, causal SDPA, out-proj.

Sharding over 8 cores: core i -> batch b = i//2, head-half p = i%2
(q-heads 8p..8p+7, kv-heads 2p, 2p+1 -- KV groups intact). Each core computes
its 8 heads' attention output O^T (feature-major), pairs AllGather O^T per
(q-tile, head-pair) slice, and each core computes a disjoint 512-column slice
of the out-projection.

On-chip layouts are feature-major ("transposed"): X^T, Q^T, K^T so the PE
contracts over partitions; V is token-major with a ones column appended so the
PV matmul also produces softmax row-sums (normalization happens on O^T).
Host pre-arranges x and weights into the SBUF layouts so all input DMAs are
contiguous copies.
"""
import sys

sys.path.insert(0, "/opt/trn_rl_repo")

import numpy as np
import ml_dtypes

import concourse.bacc as bacc
import concourse.mybir as mybir
import concourse.tile as tile
from concourse.bass_utils import run_bass_kernel_spmd

F32 = mybir.dt.float32
BF16 = mybir.dt.bfloat16
AF = mybir.ActivationFunctionType

N = 2048          # tokens
C = 1024          # model dim
DQ = 512          # local q out-features (8 heads x 64)
DKV = 128         # local kv out-features (2 kv heads x 64)
D = 64            # head dim
NCC = C // 128    # 8 contraction chunks
NQT = 4           # q tiles of 512
NTC = N // 128    # 16 token chunks
QK_GAIN = 1.5
ROPE_BASE = 10000.0
EXP_SCALE = QK_GAIN / np.sqrt(D).item()  # folded gain * 1/sqrt(D) = 0.1875
EPS = float(np.finfo(np.float32).eps)

# stream_shuffle mask: 32 groups of 4 partitions; rope rotation swaps
# 32-partition halves within each 64-partition head block.
ROT_MASK = list(range(8, 16)) + list(range(0, 8)) + list(range(24, 32)) + list(range(16, 24))


def _host_tables():
    inv_freq = (1.0 / (ROPE_BASE ** (np.arange(0, D, 2, dtype=np.float64) / D)))  # [32]
    t = np.arange(N, dtype=np.float64)
    ang = np.outer(inv_freq, t)  # [32, N]
    cos32 = np.cos(ang)
    sin32 = np.sin(ang)
    cosT = np.tile(cos32, (4, 1)).astype(np.float32)  # [128, N]
    sinTs = np.concatenate([-sin32, sin32, -sin32, sin32], axis=0).astype(np.float32)
    q = np.arange(128)
    trimask = (q[None, :] >= q[:, None]).astype(np.float32)  # keep q >= k
    ones33 = np.zeros((128, 33), np.float32)
    ones33[0:64, 0] = 1.0
    ones33[64:128, 32] = 1.0
    ident = np.eye(128, dtype=np.float32)
    exp33 = np.zeros((33, 64), np.float32)
    exp33[0, :] = 1.0
    exp33[32, :] = 1.0
    bf = ml_dtypes.bfloat16
    return {
        "cosT": cosT.astype(bf),
        "sinTs": sinTs.astype(bf),
        "trimask": trimask.astype(bf),
        "ones33": ones33.astype(bf),
        "ident": ident.astype(bf),
        "exp33": exp33.astype(bf),
    }


def build():
    nc = bacc.Bacc(None, target_bir_lowering=False, num_devices=8)

    # host pre-arranged: x^T token-tiled [qt, 128, cc, 512]; weights [128, cc, outfeat]
    x_ext = nc.declare_dram_parameter("x", [NQT, 128, NCC, 512], BF16, isOutput=False)
    wq_ext = nc.declare_dram_parameter("wq", [128, NCC, DQ], BF16, isOutput=False)
    wk_ext = nc.declare_dram_parameter("wk", [128, NCC, DKV], BF16, isOutput=False)
    wv_ext = nc.declare_dram_parameter("wv", [128, NCC, DKV], BF16, isOutput=False)
    wo_ext = nc.declare_dram_parameter("wo", [128, NCC, DQ], BF16, isOutput=False)
    out_ext = nc.declare_dram_parameter("out", [N, DQ], F32, isOutput=True)

    tabs = _host_tables()
    cosT_d = nc.inline_tensor(tabs["cosT"], name="cosT_d")
    sinTs_d = nc.inline_tensor(tabs["sinTs"], name="sinTs_d")
    trimask_d = nc.inline_tensor(tabs["trimask"], name="trimask_d")
    ones33_d = nc.inline_tensor(tabs["ones33"], name="ones33_d")
    ident_d = nc.inline_tensor(tabs["ident"], name="ident_d")
    exp33_d = nc.inline_tensor(tabs["exp33"], name="exp33_d")

    with tile.TileContext(nc) as tc:
        with (
            tc.tile_pool(name="dram", bufs=1, space="DRAM") as dram,
            tc.tile_pool(name="persist", bufs=1) as ps,
            tc.tile_pool(name="work", bufs=3) as wk,
        ):
            # ---- persistent SBUF tensors ----
            xT = ps.tile([128, NCC, N], BF16, name="xT")          # X^T chunks
            wq_sb = ps.tile([128, NCC, DQ], BF16, name="wq_sb")
            wk_sb = ps.tile([128, NCC, DKV], BF16, name="wk_sb")
            wv_sb = ps.tile([128, NCC, DKV], BF16, name="wv_sb")
            wo_sb = ps.tile([128, NCC, DQ], BF16, name="wo_sb")
            cosT = ps.tile([128, N], BF16, name="cosT")
            sinTs = ps.tile([128, N], BF16, name="sinTs")
            trimask = ps.tile([128, 128], BF16, name="trimask")
            ones33 = ps.tile([128, 33], BF16, name="ones33")
            ident = ps.tile([128, 128], BF16, name="ident")
            exp33 = ps.tile([33, 64], BF16, name="exp33")
            eps_sb = ps.tile([128, 1], F32, name="eps_sb")
            kTdA = ps.tile([128, N], BF16, name="kTdA")           # kv head A dup'd
            kTdB = ps.tile([128, N], BF16, name="kTdB")
            v_sb = ps.tile([128, NTC, 130], BF16, name="v_sb")    # [V_A|1|V_B|1]
            oT = ps.tile([128, 4, N], BF16, name="oT")            # own O^T (normed)

            # ---- phase A: stage inputs; x token-tiled so projections can
            # start after the first 1MB tile lands ----
            for qt in range(NQT):
                eng = nc.sync if qt % 2 == 0 else nc.scalar
                eng.dma_start(
                    out=xT[:, :, qt * 512:(qt + 1) * 512], in_=x_ext[qt],
                )
            nc.gpsimd.dma_start(out=wq_sb[:], in_=wq_ext[:])
            nc.gpsimd.dma_start(out=wk_sb[:], in_=wk_ext[:])
            nc.gpsimd.dma_start(out=ones33[:], in_=ones33_d[:])
            nc.gpsimd.dma_start(out=cosT[:], in_=cosT_d[:])
            nc.gpsimd.dma_start(out=sinTs[:], in_=sinTs_d[:])
            nc.gpsimd.dma_start(out=exp33[:], in_=exp33_d[:])
            nc.gpsimd.dma_start(out=wv_sb[:], in_=wv_ext[:])
            nc.gpsimd.dma_start(out=ident[:], in_=ident_d[:])
            nc.gpsimd.dma_start(out=trimask[:], in_=trimask_d[:])
            nc.gpsimd.dma_start(out=wo_sb[:], in_=wo_ext[:])
            nc.gpsimd.memset(eps_sb[:], EPS)
            nc.gpsimd.memset(v_sb[:, :, 64:65], 1.0)
            nc.gpsimd.memset(v_sb[:, :, 129:130], 1.0)

            # ---- unified psum/work pools (tags shared across phases for overlap) ----
            with (
                tc.tile_pool(name="u_psum", bufs=1, space="PSUM") as up,
                tc.tile_pool(name="u_sbuf", bufs=3) as bs,
            ):
                qT_raw = bs.tile([128, 4, N], BF16, name="qT_raw", bufs=1)
                kT_raw = bs.tile([128, N], BF16, name="kT_raw", bufs=1)

                def proj(w_ap, dst):
                    # dst [128, N]; w_ap [128, cc, 128] weight slice
                    for qp in range(NQT // 2):
                        pp = up.tile([128, 2, 512], F32, tag=("mm" if qp % 2 == 0 else "o"), bufs=2)
                        for h in range(2):
                            qt = 2 * qp + h
                            for cc in range(NCC):
                                nc.tensor.matmul(
                                    pp[:, h, :], w_ap[:, cc, :], xT[:, cc, qt * 512:(qt + 1) * 512],
                                    start=(cc == 0), stop=(cc == NCC - 1),
                                )
                        nc.vector.tensor_copy(
                            dst[:, qp * 1024:(qp + 1) * 1024],
                            pp.rearrange("p a b -> p (a b)"),
                        )

                def sq_fill(sq, src, hh):
                    nc.vector.tensor_mul(
                        sq[:, hh * 1024:(hh + 1) * 1024],
                        src[:, hh * 1024:(hh + 1) * 1024],
                        src[:, hh * 1024:(hh + 1) * 1024],
                    )

                def rope_sq(src):
                    sq = bs.tile([128, N], BF16, tag="sq", bufs=2)
                    sq_fill(sq, src, 0)
                    sq_fill(sq, src, 1)
                    return sq

                def rope_ln(sq):
                    # rms denominators: rr2 rows 0/32 hold per-token 1/rms for A/B heads
                    lnv = bs.tile([33, N], F32, tag="lnv", bufs=1)
                    for qt in range(NQT):
                        msp = up.tile([33, 512], F32, tag="mm", bufs=2)
                        nc.tensor.matmul(
                            msp[:], ones33[:],
                            sq[:, qt * 512:(qt + 1) * 512], start=True, stop=True,
                        )
                        nc.scalar.activation(
                            lnv[:, qt * 512:(qt + 1) * 512], msp[:],
                            AF.Ln, bias=eps_sb[0:33, :], scale=1.0 / D,
                        )
                    rr2 = bs.tile([33, N], BF16, tag="rr2", bufs=2)
                    nc.scalar.activation(rr2[:], lnv[:], AF.Exp, scale=-0.5)
                    return rr2

                def rope_apply(src, rr2, dsts):
                    rot = bs.tile([128, N], BF16, tag="rot", bufs=2)
                    nc.vector.tensor_copy(rot[0:32, :], src[32:64, :])
                    nc.vector.tensor_copy(rot[32:64, :], src[0:32, :])
                    nc.vector.tensor_copy(rot[64:96, :], src[96:128, :])
                    nc.vector.tensor_copy(rot[96:128, :], src[64:96, :])
                    t1 = bs.tile([128, N], BF16, tag="t1", bufs=2)
                    nc.vector.tensor_mul(t1[:], src, cosT[:])
                    nc.vector.tensor_mul(rot[:], rot[:], sinTs[:])
                    nc.vector.tensor_add(t1[:], t1[:], rot[:])
                    # apply r (per head, per token) via expander broadcast
                    for qt in range(NQT):
                        rbp = up.tile([128, 512], F32, tag="mm", bufs=2)
                        nc.tensor.matmul(
                            rbp[0:64, :], exp33[0:1, :], rr2[0:1, qt * 512:(qt + 1) * 512],
                            start=True, stop=True,
                        )
                        nc.tensor.matmul(
                            rbp[64:128, :], exp33[32:33, :], rr2[32:33, qt * 512:(qt + 1) * 512],
                            start=True, stop=True,
                        )
                        for dst in dsts:
                            nc.vector.tensor_mul(
                                dst[:, qt * 512:(qt + 1) * 512],
                                t1[:, qt * 512:(qt + 1) * 512], rbp[:],
                            )

                vT = bs.tile([128, N], BF16, tag="og", bufs=2)

                def emit_vtrans(group):
                    # transpose V token-chunks 4*group .. 4*group+3 into v_sb
                    for tcix in range(4 * group, 4 * group + 4):
                        pv = up.tile([128, 128], BF16, tag=("mm" if tcix % 2 == 0 else "o"), bufs=2)
                        nc.tensor.transpose(pv[:], vT[:, tcix * 128:(tcix + 1) * 128], ident[:])
                        nc.vector.tensor_copy(v_sb[:, tcix, 0:64], pv[:, 0:64])
                        nc.vector.tensor_copy(v_sb[:, tcix, 65:129], pv[:, 64:128])

                # ---- phase D: attention (j outer) + per-(j,m) AllGather + out-proj ----
                qTf = qT_raw
                cc_ins = [[dram.tile([128, 512], BF16, name=f"cc_in{j}_{m}") for m in range(4)] for j in range(NQT)]
                cc_outs = [[dram.tile([2, 128, 512], BF16, name=f"cc_out{j}_{m}") for m in range(4)] for j in range(NQT)]
                og_tiles = {}

                def emit_outproj(jo, tts):
                    og = og_tiles[jo]
                    for tt in tts:
                        tcix = jo * 4 + tt
                        po = up.tile([128, 512], F32, tag="o", bufs=2, name=f"po{jo}_{tt}")
                        for rc in range(8):
                            nc.tensor.matmul(
                                po[:], og[:, rc, tt * 128:(tt + 1) * 128],
                                wo_sb[:, rc, :], start=(rc == 0), stop=(rc == 7),
                            )
                        ev = bs.tile([128, 512], F32, tag="ev", bufs=2, name=f"ev{jo}_{tt}")
                        nc.vector.tensor_copy(ev[:], po[:])
                        nc.sync.dma_start(
                            out=out_ext[tcix * 128:(tcix + 1) * 128, :], in_=ev[:]
                        )

                def emit_attention(j, m, defer_pv=False):
                    kT = kTdA if m < 2 else kTdB
                    vslot = 0 if m < 2 else 65
                    oab = up.tile([65, 2, 512], F32, tag="o", bufs=2)
                    nkc = 4 * (j + 1)

                    def emit_scores(kc):
                        i = kc - 4 * j
                        off = max(0, 128 * i)
                        w = 512 - off
                        q0 = 512 * j + off
                        sAB = up.tile([128, 2, 512], F32, tag="mm", bufs=2, name=f"sAB{kc}")
                        nc.tensor.matmul(
                            sAB[:, 0, 0:w], kT[0:64, kc * 128:(kc + 1) * 128],
                            qTf[0:64, m, q0:q0 + w], start=True, stop=True,
                            tile_position=(0, 0),
                        )
                        nc.tensor.matmul(
                            sAB[:, 1, 0:w], kT[64:128, kc * 128:(kc + 1) * 128],
                            qTf[64:128, m, q0:q0 + w], start=True, stop=True,
                            tile_position=(64, 0),
                        )
                        pAB = bs.tile([128, 2, 512], BF16, tag="pAB", bufs=5, name=f"pAB{kc}")
                        nc.scalar.activation(
                            pAB[:, :, 0:w], sAB[:, :, 0:w], AF.Exp, scale=EXP_SCALE
                        )
                        if i >= 0:
                            nc.vector.tensor_mul(
                                pAB[:, :, 0:128], pAB[:, :, 0:128],
                                trimask.rearrange("p (a b) -> p a b", a=1).broadcast_to([128, 2, 128]),
                            )
                        return pAB

                    def emit_pv(kc, pAB):
                        i = kc - 4 * j
                        off = max(0, 128 * i)
                        w = 512 - off
                        nc.tensor.matmul(
                            oab[:, 0, off:512], v_sb[:, kc, vslot:vslot + 65],
                            pAB[:, 0, 0:w], start=(kc == 0), stop=(kc == nkc - 1),
                            skip_group_check=True,
                        )
                        nc.tensor.matmul(
                            oab[:, 1, off:512], v_sb[:, kc, vslot:vslot + 65],
                            pAB[:, 1, 0:w], start=(kc == 0), stop=(kc == nkc - 1),
                            skip_group_check=True,
                        )

                    if defer_pv:
                        # emit all scores+exp now; PV + normalization resume
                        # via the returned closure (lets V transposes slot in
                        # between on the PE stream)
                        pabs = [emit_scores(kc) for kc in range(nkc)]

                        def finish():
                            for kc in range(nkc):
                                emit_pv(kc, pabs[kc])
                            _norm_and_gather(j, m, oab)
                        return finish
                    staged = []
                    for kc in range(nkc):
                        staged.append((kc, emit_scores(kc)))
                        if len(staged) == 2:
                            for kcx, px in staged:
                                emit_pv(kcx, px)
                            staged = []
                    for kcx, px in staged:
                        emit_pv(kcx, px)
                    _norm_and_gather(j, m, oab)

                def _norm_and_gather(j, m, oab):
                    # normalize from PSUM directly: r = 1/rowsum, broadcast,
                    # multiply into oT (psum-direct, no staging copies)
                    ssum = bs.tile([1, 1024], F32, tag="ssum", bufs=2)
                    nc.vector.tensor_copy(ssum[:], oab[64:65, :, :])
                    rrf = bs.tile([1, 1024], F32, tag="rrf", bufs=2)
                    nc.vector.reciprocal_approx_fast(rrf[:], ssum[:])
                    rrb = bs.tile([1, 1024], BF16, tag="rrb", bufs=2)
                    nc.vector.tensor_copy(rrb[:], rrf[:])
                    rbs = bs.tile([64, 1024], BF16, tag="rbs", bufs=2)
                    nc.gpsimd.partition_broadcast(rbs[:], rrb[:], channels=64)
                    nc.vector.tensor_mul(
                        oT[0:64, m, 512 * j:512 * (j + 1)], oab[0:64, 0, :], rbs[:, 0:512]
                    )
                    nc.vector.tensor_mul(
                        oT[64:128, m, 512 * j:512 * (j + 1)], oab[0:64, 1, :], rbs[:, 512:1024]
                    )
                    # per-(j,m) AllGather of this O^T slice
                    nc.sync.dma_start(
                        out=cc_ins[j][m][:], in_=oT[:, m, 512 * j:512 * (j + 1)]
                    )
                    nc.gpsimd.collective_compute(
                        "AllGather",
                        mybir.AluOpType.bypass,
                        replica_groups=[[0, 1], [2, 3], [4, 5], [6, 7]],
                        ins=[cc_ins[j][m].opt()],
                        outs=[cc_outs[j][m].opt()],
                    )
                    if m == 0:
                        og_tiles[j] = bs.tile([128, 8, 512], BF16, tag="og", bufs=2, name=f"og{j}")
                    og = og_tiles[j]
                    for r in range(2):
                        nc.sync.dma_start(
                            out=og[:, r * 4 + m, :], in_=cc_outs[j][m][r],
                        )

                # Head: Q-proj m0 and K-proj interleaved at qp granularity so
                # the rms chain (sq halves -> msp -> Ln) starts as soon as
                # the first half of each projection lands.
                for qp in range(NQT // 2):
                    for wi, (w_ap, dst) in enumerate(
                        ((wq_sb[:, :, 0:128], qT_raw[:, 0, :]), (wk_sb, kT_raw[:]))
                    ):
                        pp = up.tile([128, 2, 512], F32, tag=("mm" if (2 * qp + wi) % 2 == 0 else "o"), bufs=2)
                        for h in range(2):
                            qt = 2 * qp + h
                            for cc in range(NCC):
                                nc.tensor.matmul(
                                    pp[:, h, :], w_ap[:, cc, :], xT[:, cc, qt * 512:(qt + 1) * 512],
                                    start=(cc == 0), stop=(cc == NCC - 1),
                                )
                        nc.vector.tensor_copy(
                            dst[:, qp * 1024:(qp + 1) * 1024],
                            pp.rearrange("p a b -> p (a b)"),
                        )
                    if qp == 0:
                        sq_q0 = bs.tile([128, N], BF16, tag="sq", bufs=2)
                        sq_fill(sq_q0, qT_raw[:, 0, :], 0)
                        sq_k = bs.tile([128, N], BF16, tag="sq", bufs=2)
                        sq_fill(sq_k, kT_raw[:], 0)
                    else:
                        sq_fill(sq_q0, qT_raw[:, 0, :], 1)
                        sq_fill(sq_k, kT_raw[:], 1)
                rr2_q0 = rope_ln(sq_q0)
                rr2_k = rope_ln(sq_k)
                rope_apply(qT_raw[:, 0, :], rr2_q0, [qT_raw[:, 0, :]])
                rope_apply(kT_raw[:], rr2_k, [kTdA[:]])
                nc.vector.tensor_copy(kTdB[0:64, :], kTdA[64:128, :])
                nc.vector.tensor_copy(kTdB[64:128, :], kTdA[64:128, :])
                nc.vector.tensor_copy(kTdA[64:128, :], kTdA[0:64, :])
                att00_finish = emit_attention(0, 0, defer_pv=True)
                proj(wv_sb, vT[:])
                emit_vtrans(0)
                att00_finish()
                # j=0 (and early j=1) attention interleaved with remaining Q
                # proj + rope; V transposes slot between rope_ln and
                # rope_apply to cover the rms scalar chain latency; j=1
                # blocks for already-roped m fill scalar while the PE
                # projects the next q chunk.
                for m in range(1, 4):
                    proj(wq_sb[:, :, m * 128:(m + 1) * 128], qT_raw[:, m, :])
                    sq = rope_sq(qT_raw[:, m, :])
                    rr2 = rope_ln(sq)
                    emit_vtrans(m)
                    rope_apply(qT_raw[:, m, :], rr2, [qT_raw[:, m, :]])
                    emit_attention(0, m)
                    emit_attention(1, m - 1)
                emit_attention(1, 3)
                emit_outproj(0, [0, 1])
                for m in range(4):
                    emit_attention(2, m)
                    emit_outproj(0, [2, 3]) if m == 0 else emit_outproj(1, [m - 1])
                for m in range(4):
                    emit_attention(3, m)
                    emit_outproj(1, [3]) if m == 0 else emit_outproj(2, [m - 1])
                emit_outproj(2, [3])
                emit_outproj(NQT - 1, range(4))

    nc.finalize()
    return nc


_NC_CACHE = None


def _get_nc():
    global _NC_CACHE
    if _NC_CACHE is None:
        _NC_CACHE = build()
    return _NC_CACHE


def _make_in_maps(inputs):
    x = np.asarray(inputs["x"], dtype=np.float32)
    wq = np.asarray(inputs["wq"], dtype=np.float32)
    wk = np.asarray(inputs["wk"], dtype=np.float32)
    wv = np.asarray(inputs["wv"], dtype=np.float32)
    wo = np.asarray(inputs["wo"], dtype=np.float32)
    bf = ml_dtypes.bfloat16

    def warr(w):  # [1024, F] -> [128, 8, F]
        return np.ascontiguousarray(w.reshape(NCC, 128, -1).transpose(1, 0, 2).astype(bf))

    in_maps = []
    for i in range(8):
        b, p = i // 2, i % 2
        xb = x[b]  # [N, C]
        xT = xb.T.reshape(NCC, 128, N).transpose(1, 0, 2)  # [128, cc, n]
        x4 = np.ascontiguousarray(
            xT.reshape(128, NCC, NQT, 512).transpose(2, 0, 1, 3).astype(bf)
        )  # [qt, 128, cc, 512]
        in_maps.append({
            "x": x4,
            "wq": warr(wq[:, p * DQ:(p + 1) * DQ]),
            "wk": warr(wk[:, p * DKV:(p + 1) * DKV]),
            "wv": warr(wv[:, p * DKV:(p + 1) * DKV]),
            "wo": warr(wo[:, p * DQ:(p + 1) * DQ]),
        })
    return in_maps


def kernel(x, wq, wk, wv, wo):
    x = np.asarray(x, dtype=np.float32)
    B = x.shape[0]
    nc = _get_nc()
    in_maps = _make_in_maps({"x": x, "wq": wq, "wk": wk, "wv": wv, "wo": wo})
    res = run_bass_kernel_spmd(nc, in_maps, core_ids=list(range(8)))
    out = np.empty((B, N, C), dtype=np.float32)
    for b in range(B):
        out[b, :, 0:DQ] = res.results[2 * b]["out"]
        out[b, :, DQ:C] = res.results[2 * b + 1]["out"]
    return out


if __name__ == "__main__":
    rng = np.random.default_rng(0)
    ins = {
        "x": rng.standard_normal((4, N, C), dtype=np.float32),
        "wq": (rng.standard_normal((C, C), dtype=np.float32) * 0.02),
        "wk": (rng.standard_normal((C, 256), dtype=np.float32) * 0.02),
        "wv": (rng.standard_normal((C, 256), dtype=np.float32) * 0.02),
        "wo": (rng.standard_normal((C, C), dtype=np.float32) * 0.02),
    }
    y = kernel(**ins)
    print("out", y.shape, y.dtype, np.abs(y).mean())
